# revision 27
# baseline (speedup 1.0000x reference)
"""LSTM encoder with EOS-freeze for Trainium2, data-parallel over batch on 8 cores.

Strategy
--------
Inputs are one-hot, so x @ Wi is a row-gather of Wi done with indirect DMA on
device. The recurrent h @ Wh runs on the tensor engine with Wh as 64 fp16
[128,128] stationary tiles and h.T chunks as the [128,16] moving operand,
producing z transposed: PSUM [128 partitions = z-feature % 128, 16*tile + b].
Gates are reordered (g, i, f, o) host-side, one PSUM bank per gate, so each
gate's activation starts as soon as its own 16 recurrent matmuls close instead
of waiting for all 64 — the elementwise chain overlaps the tensor engine.

Gathered x rows are packed 4 steps per 2048-column slice at partition offsets
{0,32,64,96} (the valid PE tile_position row bases for a 16-row stationary).
This spreads gather/preload DMAs across 64 partitions instead of 16 (4x DMA
bandwidth), shrinks SBUF so a 64-step loop body fits (two For_i barriers plus
an unrolled epilogue block instead of 15 boundaries), and batches the
per-block gathers into 16 DGE instructions. Block 0's rows are gathered
host-side and loaded with a few direct DMAs (smallest chunk first) so the
gpsimd queue is empty before the loop-entry barrier and the first matmuls
start ~13us into the kernel. The epilogue block issues no gathers, letting
the DMA queues drain during, not after, the final 64 steps.

The EOS freeze is handled without any per-step masking: sequences are
independent, so the kernel runs the unmasked recurrence and streams per-step
(c, h) snapshots to DRAM; the frozen value for sequence b is the snapshot at
its first-EOS step, selected during unshard.
"""

import numpy as np

try:
    import concourse  # noqa: F401
except ImportError:
    import sys

    sys.path.insert(0, "/opt/trn_rl_repo")

from contextlib import ExitStack

import concourse.bass as bass
import concourse.tile as tile
from concourse import bacc
from concourse import mybir
from concourse.bass import ds
from concourse.bass_utils import run_bass_kernel_spmd

dt = mybir.dt
Alu = mybir.AluOpType
Act = mybir.ActivationFunctionType

EOS_ID = 1
HID = 512
BATCH, SEQ, VOCAB = 128, 256, 1024
GATES = 4 * HID  # 2048
NCORES = 8
BLOC = BATCH // NCORES  # 16 sequences per core
NT = GATES // 128  # 16 feature tiles of z
NK = HID // 128  # 4 contraction chunks
SPG = 4  # steps packed per ST column-slice (partition offsets 0/32/64/96)
BODY = 64  # steps per For_i iteration
NGRP = BODY // SPG  # gather groups per block

# Collect profiling info when True (set by test.py; adds trace overhead).
TRACE = False
LAST_RESULTS = None  # BassKernelResults of the last run, for test.py

_PROGRAM = None


def _build_program(seq=SEQ, body=BODY):
    ngrp = body // SPG
    nc = bacc.Bacc("TRN2", debug=False, detect_race_conditions=False)

    wi = nc.declare_dram_parameter("wi", [VOCAB, GATES], dt.float16, isOutput=False)
    ident = nc.declare_dram_parameter("ident", [128, BLOC], dt.float16, isOutput=False)
    wh = nc.declare_dram_parameter("wh", [128, NK * NT * 128], dt.float16, isOutput=False)
    # tok4[32u+b, j] = token of sequence b at step 4j+u (b<16; other rows 0).
    tok4 = nc.declare_dram_parameter(
        "tok4", [128, (seq + body) // SPG], dt.int32, isOutput=False
    )
    # Block 0's gathered Wi rows, prepared host-side (one direct DMA).
    st0 = nc.declare_dram_parameter("st0", [128, ngrp * GATES], dt.float16, isOutput=False)
    c_traj = nc.declare_dram_parameter("c_traj", [seq * 128, 64], dt.float16, isOutput=True)
    h_traj = nc.declare_dram_parameter("h_traj", [seq * 128, 64], dt.float16, isOutput=True)

    with tile.TileContext(nc) as tc, ExitStack() as ctx:
        pool = lambda name, bufs, **kw: ctx.enter_context(
            tc.tile_pool(name=name, bufs=bufs, **kw)
        )
        whp = pool("whp", 1)
        tokp = pool("tokp", 1)
        stp = pool("stp", 1)
        hp = pool("hp", 1)
        cp = pool("cp", 1)
        zp_pool = pool("zp", 2, space="PSUM")
        sp = pool("sp", 2)
        gp = pool("gp", 2)
        ap_ = pool("ap", 2)
        bp = pool("bp", 2)
        tp = pool("tp", 2)

        # One big tile holding all gather groups; subtile dep tracking scopes
        # reads/writes to the per-group 2048-column slices. Load block 0 in a
        # few chunks, smallest first and ahead of the 2MB wh transfer, so the
        # first steps' matmuls start as early as possible — but no more DMAs
        # than free DMA semaphores, or the sync queue blocks on semaphore
        # reuse before the loop-entry barrier.
        ST = stp.tile([128, ngrp * GATES], dt.float16, name="st_all")
        id_sb = tokp.tile([128, BLOC], dt.float16, name="id_sb")
        nc.sync.dma_start(out=id_sb[:], in_=ident[:, :])
        nc.sync.dma_start(out=ST[:, 0:GATES], in_=st0[:, 0:GATES])
        wh_sb = whp.tile([128, NK * NT * 128], dt.float16, name="wh_sb")
        nc.sync.dma_start(out=wh_sb[:], in_=wh[:, :])
        tok_cur = tokp.tile([128, ngrp], dt.int32, name="tok_cur")
        nc.sync.dma_start(out=tok_cur[:], in_=tok4[:, 0:ngrp])
        bounds = [1, 2, (ngrp + 2) // 2, ngrp]
        for j0, j1 in zip(bounds, bounds[1:]):
            nc.sync.dma_start(
                out=ST[:, j0 * GATES : j1 * GATES],
                in_=st0[:, j0 * GATES : j1 * GATES],
            )
        H = [hp.tile([128, 64], dt.float16, name=f"h{s}", tag=f"h{s}") for s in range(body)]
        C = [cp.tile([128, 64], dt.float16, name=f"c{s}", tag=f"c{s}") for s in range(body)]

        nc.vector.memset(H[body - 1][:], 0.0)
        nc.vector.memset(C[body - 1][:], 0.0)

        def gather_xp(j, nj=1):
            # Gather wi rows for groups j..j+nj-1 (4 steps x 16 sequences per
            # group; rows 16..31 of each 32-partition group are dummy token
            # 0) — one row per partition per offset column. tok_cur always
            # holds the token columns for the block being prefetched.
            nc.gpsimd.indirect_dma_start(
                out=ST[:, j * GATES : (j + nj) * GATES],
                out_offset=None,
                in_=wi[:, :],
                in_offset=bass.IndirectOffsetOnAxis(ap=tok_cur[:, j : j + nj], axis=0),
            )

        def step(jv, s, gather=True):
            u, j = s % SPG, s // SPG
            hprev = H[(s - 1) % body]
            cprev = C[(s - 1) % body]
            # One PSUM bank per gate so each gate's accumulation group closes
            # after its own 16 matmuls and its activation overlaps the
            # remaining gates' matmuls. Gate order in z columns: g, i, f, o.
            Z = [
                zp_pool.tile([128, 64], dt.float32, name=f"z{q}", tag=f"z{q}")
                for q in range(4)
            ]
            # x@Wi enters PSUM via PE transpose of the gathered rows: these
            # matmuls need no h, so they overlap the previous step's tail.
            for q in range(4):
                for t in range(4):
                    tg = 4 * q + t
                    # start=True on the first matmul clears the bank's
                    # has_written bits; later matmuls join the group.
                    nc.tensor.matmul(
                        out=Z[q][:, 16 * t : 16 * t + 16],
                        lhsT=ST[
                            32 * u : 32 * u + BLOC,
                            j * GATES + 128 * tg : j * GATES + 128 * tg + 128,
                        ],
                        rhs=id_sb[32 * u : 32 * u + BLOC, :],
                        start=(t == 0),
                        stop=False,
                        tile_position=(32 * u, 0),
                    )
            for q in range(4):
                for t in range(4):
                    tg = 4 * q + t
                    for k in range(NK):
                        nc.tensor.matmul(
                            out=Z[q][:, 16 * t : 16 * t + 16],
                            lhsT=wh_sb[:, (k * NT + tg) * 128 : (k * NT + tg) * 128 + 128],
                            rhs=hprev[:, 16 * k : 16 * k + 16],
                            start=False,
                            stop=(t == 3 and k == NK - 1),
                        )
            TG = gp.tile([128, 64], dt.float16, name="TG", tag="TG")
            nc.scalar.activation(out=TG[:], in_=Z[0][:], func=Act.Tanh)
            SI = sp.tile([128, 64], dt.float16, name="SI", tag="SI")
            nc.scalar.activation(out=SI[:], in_=Z[1][:], func=Act.Sigmoid)
            SF = sp.tile([128, 64], dt.float16, name="SF", tag="SF")
            nc.scalar.activation(out=SF[:], in_=Z[2][:], func=Act.Sigmoid)
            SO = sp.tile([128, 64], dt.float16, name="SO", tag="SO")
            nc.scalar.activation(out=SO[:], in_=Z[3][:], func=Act.Sigmoid)
            A = ap_.tile([128, 64], dt.float16, name="A", tag="A")
            nc.vector.tensor_tensor(out=A[:], in0=SI[:], in1=TG[:], op=Alu.mult)
            B = bp.tile([128, 64], dt.float16, name="B", tag="B")
            nc.vector.tensor_tensor(out=B[:], in0=SF[:], in1=cprev[:], op=Alu.mult)
            cs = C[s]
            nc.vector.tensor_tensor(out=cs[:], in0=A[:], in1=B[:], op=Alu.add)
            T = tp.tile([128, 64], dt.float16, name="T", tag="T")
            nc.scalar.activation(out=T[:], in_=cs[:], func=Act.Tanh)
            hs = H[s]
            nc.vector.tensor_tensor(out=hs[:], in0=SO[:], in1=T[:], op=Alu.mult)

            nc.sync.dma_start(out=c_traj[ds((jv * SPG + s) * 128, 128), :], in_=cs[:])
            nc.sync.dma_start(out=h_traj[ds((jv * SPG + s) * 128, 128), :], in_=hs[:])
            # After the last step of group j has read its ST slice, prefetch
            # that slice for the next block.
            if u == SPG - 1 and gather:
                gather_xp(j)

        # The loop runs all blocks except the last; the unrolled epilogue
        # block issues no gathers or token DMAs, so the DMA queues drain
        # during (not after) the final 64 steps.
        with tc.For_i(
            0, (seq - body) // SPG, body // SPG,
            hint_engines=(mybir.EngineType.PE,), staggered_reset=False,
        ) as jv:
            # Stage the NEXT block's token columns; in-loop gathers prefetch
            # for block i+1 while this block computes.
            nc.sync.dma_start(out=tok_cur[:], in_=tok4[:, ds(jv + ngrp, ngrp)])
            for s in range(body):
                step(jv, s)
        for s in range(body):
            step((seq - body) // SPG, s, gather=False)

    nc.finalize()
    return nc


def _get_program():
    global _PROGRAM
    if _PROGRAM is None:
        _PROGRAM = _build_program()
    return _PROGRAM


def _prep_host(inputs, Wi, Wh, b):
    tokens = np.argmax(inputs, axis=-1).astype(np.int32)  # [B, T]
    eos = inputs[:, :, EOS_ID] > 0.5
    any_eos = eos.any(axis=1)
    t_star = np.where(any_eos, eos.argmax(axis=1), SEQ - 1).astype(np.int64)

    # Gate reorder (g, i, f, o): gates whose results are needed earliest in
    # the elementwise chain close their PSUM banks first; o closes last and
    # has the shortest tail (h = sig(o) * tanh(c)).
    perm = np.concatenate(
        [np.arange(1024, 1536), np.arange(0, 512), np.arange(512, 1024), np.arange(1536, 2048)]
    )
    Wi_re = (Wi.astype(np.float32) + b.astype(np.float32)[None, :])[:, perm]
    Wh_re = Wh.astype(np.float32)[:, perm]

    # wi rows stay in z-feature order (gate-permuted only); wh tile k*16+t
    # holds Wh_re[128k:128k+128, 128t:128t+128], stored partition-major.
    Wi_dev = np.ascontiguousarray(Wi_re).astype(np.float16)
    # Partition-major: wh[kr, (k*NT+t)*128 + p] = Wh_re[128k+kr, 128t+p]
    Wh_dev = np.ascontiguousarray(
        Wh_re.reshape(NK, 128, NT, 128).transpose(1, 0, 2, 3).reshape(128, NK * NT * 128)
    ).astype(np.float16)
    return tokens, t_star, Wi_dev, Wh_dev


def _pack4(arr16):
    """[16, 4*n, ...] -> [128, n, ...] with step 4j+u at partition 32u+b."""
    n4 = arr16.shape[1]
    n = n4 // SPG
    out = np.zeros((128, n) + arr16.shape[2:], arr16.dtype)
    for u in range(SPG):
        out[32 * u : 32 * u + BLOC] = arr16[:, u::SPG]
    return out


def kernel(inputs, Wi, Wh, b):
    global LAST_RESULTS
    inputs = np.asarray(inputs)
    Wi = np.asarray(Wi)
    Wh = np.asarray(Wh)
    b = np.asarray(b)

    tokens, t_star, Wi_dev, Wh_dev = _prep_host(inputs, Wi, Wh, b)

    id_rep = np.zeros((128, BLOC), np.float16)
    for u in range(SPG):
        id_rep[32 * u : 32 * u + BLOC] = np.eye(BLOC, dtype=np.float16)

    in_maps = []
    for n in range(NCORES):
        tokc = tokens[BLOC * n : BLOC * (n + 1)]
        tok_pad = np.concatenate([tokc, np.zeros((BLOC, BODY), np.int32)], axis=1)
        tok4 = _pack4(tok_pad)
        st0 = _pack4(Wi_dev[tokc[:, 0:BODY]]).reshape(128, (BODY // SPG) * GATES)
        in_maps.append(
            {
                "wi": Wi_dev,
                "wh": Wh_dev,
                "tok4": np.ascontiguousarray(tok4),
                "ident": id_rep,
                "st0": np.ascontiguousarray(st0),
            }
        )

    nc = _get_program()
    res = run_bass_kernel_spmd(nc, in_maps, list(range(NCORES)), trace=TRACE)
    LAST_RESULTS = res

    c_out = np.zeros((BATCH, HID), np.float32)
    h_out = np.zeros((BATCH, HID), np.float32)
    for n in range(NCORES):
        ct = res.results[n]["c_traj"].reshape(SEQ, 128, 64).astype(np.float32)
        ht = res.results[n]["h_traj"].reshape(SEQ, 128, 64).astype(np.float32)
        for bl in range(BLOC):
            g = BLOC * n + bl
            t = int(t_star[g])
            c_out[g] = ct[t][:, bl::BLOC].T.reshape(HID)
            h_out[g] = ht[t][:, bl::BLOC].T.reshape(HID)
    return (c_out, h_out)


# revision 28
# speedup vs baseline: 1.0008x; 1.0008x over previous
"""LSTM encoder with EOS-freeze for Trainium2, data-parallel over batch on 8 cores.

Strategy
--------
Inputs are one-hot, so x @ Wi is a row-gather of Wi done with indirect DMA on
device. The recurrent h @ Wh runs on the tensor engine with Wh as 64 fp16
[128,128] stationary tiles and h.T chunks as the [128,16] moving operand,
producing z transposed: PSUM [128 partitions = z-feature % 128, 16*tile + b].
Gates are reordered (g, i, f, o) host-side, one PSUM bank per gate, so each
gate's activation starts as soon as its own 16 recurrent matmuls close instead
of waiting for all 64 — the elementwise chain overlaps the tensor engine.

Gathered x rows are packed 4 steps per 2048-column slice at partition offsets
{0,32,64,96} (the valid PE tile_position row bases for a 16-row stationary).
This spreads gather/preload DMAs across 64 partitions instead of 16 (4x DMA
bandwidth), shrinks SBUF so a 64-step loop body fits (two For_i barriers plus
an unrolled epilogue block instead of 15 boundaries), and batches the
per-block gathers into 16 DGE instructions. Block 0's rows are gathered
host-side and loaded with a few direct DMAs (smallest chunk first) so the
gpsimd queue is empty before the loop-entry barrier and the first matmuls
start ~13us into the kernel. The epilogue block issues no gathers, letting
the DMA queues drain during, not after, the final 64 steps.

The EOS freeze is handled without any per-step masking: sequences are
independent, so the kernel runs the unmasked recurrence and streams per-step
(c, h) snapshots to DRAM; the frozen value for sequence b is the snapshot at
its first-EOS step, selected during unshard.
"""

import numpy as np

try:
    import concourse  # noqa: F401
except ImportError:
    import sys

    sys.path.insert(0, "/opt/trn_rl_repo")

from contextlib import ExitStack

import concourse.bass as bass
import concourse.tile as tile
from concourse import bacc
from concourse import mybir
from concourse.bass import ds
from concourse.bass_utils import run_bass_kernel_spmd

dt = mybir.dt
Alu = mybir.AluOpType
Act = mybir.ActivationFunctionType

EOS_ID = 1
HID = 512
BATCH, SEQ, VOCAB = 128, 256, 1024
GATES = 4 * HID  # 2048
NCORES = 8
BLOC = BATCH // NCORES  # 16 sequences per core
NT = GATES // 128  # 16 feature tiles of z
NK = HID // 128  # 4 contraction chunks
SPG = 4  # steps packed per ST column-slice (partition offsets 0/32/64/96)
BODY = 64  # steps per For_i iteration
NGRP = BODY // SPG  # gather groups per block

# Collect profiling info when True (set by test.py; adds trace overhead).
TRACE = False
LAST_RESULTS = None  # BassKernelResults of the last run, for test.py

_PROGRAM = None


def _build_program(seq=SEQ, body=BODY):
    ngrp = body // SPG
    nc = bacc.Bacc("TRN2", debug=False, detect_race_conditions=False)

    wi = nc.declare_dram_parameter("wi", [VOCAB, GATES], dt.float16, isOutput=False)
    ident = nc.declare_dram_parameter("ident", [128, BLOC], dt.float16, isOutput=False)
    wh = nc.declare_dram_parameter("wh", [128, NK * NT * 128], dt.float16, isOutput=False)
    # tok4[32u+b, j] = token of sequence b at step 4j+u (b<16; other rows 0).
    tok4 = nc.declare_dram_parameter(
        "tok4", [128, (seq + body) // SPG], dt.int32, isOutput=False
    )
    # Block 0's gathered Wi rows, prepared host-side (one direct DMA).
    st0 = nc.declare_dram_parameter("st0", [128, ngrp * GATES], dt.float16, isOutput=False)
    c_traj = nc.declare_dram_parameter("c_traj", [seq * 128, 64], dt.float16, isOutput=True)
    h_traj = nc.declare_dram_parameter("h_traj", [seq * 128, 64], dt.float16, isOutput=True)

    with tile.TileContext(nc) as tc, ExitStack() as ctx:
        pool = lambda name, bufs, **kw: ctx.enter_context(
            tc.tile_pool(name=name, bufs=bufs, **kw)
        )
        whp = pool("whp", 1)
        tokp = pool("tokp", 1)
        stp = pool("stp", 1)
        hp = pool("hp", 1)
        cp = pool("cp", 1)
        zp_pool = pool("zp", 2, space="PSUM")
        sp = pool("sp", 2)
        gp = pool("gp", 2)
        ap_ = pool("ap", 2)
        bp = pool("bp", 2)
        tp = pool("tp", 2)

        # One big tile holding all gather groups; subtile dep tracking scopes
        # reads/writes to the per-group 2048-column slices. Load block 0 in a
        # few chunks, smallest first and ahead of the 2MB wh transfer, so the
        # first steps' matmuls start as early as possible — but no more DMAs
        # than free DMA semaphores, or the sync queue blocks on semaphore
        # reuse before the loop-entry barrier.
        ST = stp.tile([128, ngrp * GATES], dt.float16, name="st_all")
        id_sb = tokp.tile([128, BLOC], dt.float16, name="id_sb")
        nc.sync.dma_start(out=id_sb[:], in_=ident[:, :])
        # Split group 0 in half: gates g,i of steps 0-3 arrive first so the
        # earliest x-matmuls start ~2.5us sooner.
        nc.sync.dma_start(out=ST[:, 0 : GATES // 2], in_=st0[:, 0 : GATES // 2])
        nc.sync.dma_start(out=ST[:, GATES // 2 : GATES], in_=st0[:, GATES // 2 : GATES])
        wh_sb = whp.tile([128, NK * NT * 128], dt.float16, name="wh_sb")
        nc.sync.dma_start(out=wh_sb[:], in_=wh[:, :])
        tok_cur = tokp.tile([128, ngrp], dt.int32, name="tok_cur")
        nc.sync.dma_start(out=tok_cur[:], in_=tok4[:, 0:ngrp])
        bounds = [1, 2, (ngrp + 2) // 2, ngrp]
        for j0, j1 in zip(bounds, bounds[1:]):
            nc.sync.dma_start(
                out=ST[:, j0 * GATES : j1 * GATES],
                in_=st0[:, j0 * GATES : j1 * GATES],
            )
        H = [hp.tile([128, 64], dt.float16, name=f"h{s}", tag=f"h{s}") for s in range(body)]
        C = [cp.tile([128, 64], dt.float16, name=f"c{s}", tag=f"c{s}") for s in range(body)]

        nc.vector.memset(H[body - 1][:], 0.0)
        nc.vector.memset(C[body - 1][:], 0.0)

        def gather_xp(j, nj=1):
            # Gather wi rows for groups j..j+nj-1 (4 steps x 16 sequences per
            # group; rows 16..31 of each 32-partition group are dummy token
            # 0) — one row per partition per offset column. tok_cur always
            # holds the token columns for the block being prefetched.
            nc.gpsimd.indirect_dma_start(
                out=ST[:, j * GATES : (j + nj) * GATES],
                out_offset=None,
                in_=wi[:, :],
                in_offset=bass.IndirectOffsetOnAxis(ap=tok_cur[:, j : j + nj], axis=0),
            )

        def step(jv, s, gather=True):
            u, j = s % SPG, s // SPG
            hprev = H[(s - 1) % body]
            cprev = C[(s - 1) % body]
            # One PSUM bank per gate so each gate's accumulation group closes
            # after its own 16 matmuls and its activation overlaps the
            # remaining gates' matmuls. Gate order in z columns: g, i, f, o.
            Z = [
                zp_pool.tile([128, 64], dt.float32, name=f"z{q}", tag=f"z{q}")
                for q in range(4)
            ]
            # x@Wi enters PSUM via PE transpose of the gathered rows: these
            # matmuls need no h, so they overlap the previous step's tail.
            for q in range(4):
                for t in range(4):
                    tg = 4 * q + t
                    # start=True on the first matmul clears the bank's
                    # has_written bits; later matmuls join the group.
                    nc.tensor.matmul(
                        out=Z[q][:, 16 * t : 16 * t + 16],
                        lhsT=ST[
                            32 * u : 32 * u + BLOC,
                            j * GATES + 128 * tg : j * GATES + 128 * tg + 128,
                        ],
                        rhs=id_sb[32 * u : 32 * u + BLOC, :],
                        start=(t == 0),
                        stop=False,
                        tile_position=(32 * u, 0),
                    )
            for q in range(4):
                for t in range(4):
                    tg = 4 * q + t
                    for k in range(NK):
                        nc.tensor.matmul(
                            out=Z[q][:, 16 * t : 16 * t + 16],
                            lhsT=wh_sb[:, (k * NT + tg) * 128 : (k * NT + tg) * 128 + 128],
                            rhs=hprev[:, 16 * k : 16 * k + 16],
                            start=False,
                            stop=(t == 3 and k == NK - 1),
                        )
            TG = gp.tile([128, 64], dt.float16, name="TG", tag="TG")
            nc.scalar.activation(out=TG[:], in_=Z[0][:], func=Act.Tanh)
            SI = sp.tile([128, 64], dt.float16, name="SI", tag="SI")
            nc.scalar.activation(out=SI[:], in_=Z[1][:], func=Act.Sigmoid)
            SF = sp.tile([128, 64], dt.float16, name="SF", tag="SF")
            nc.scalar.activation(out=SF[:], in_=Z[2][:], func=Act.Sigmoid)
            SO = sp.tile([128, 64], dt.float16, name="SO", tag="SO")
            nc.scalar.activation(out=SO[:], in_=Z[3][:], func=Act.Sigmoid)
            A = ap_.tile([128, 64], dt.float16, name="A", tag="A")
            nc.vector.tensor_tensor(out=A[:], in0=SI[:], in1=TG[:], op=Alu.mult)
            B = bp.tile([128, 64], dt.float16, name="B", tag="B")
            nc.vector.tensor_tensor(out=B[:], in0=SF[:], in1=cprev[:], op=Alu.mult)
            cs = C[s]
            nc.vector.tensor_tensor(out=cs[:], in0=A[:], in1=B[:], op=Alu.add)
            T = tp.tile([128, 64], dt.float16, name="T", tag="T")
            nc.scalar.activation(out=T[:], in_=cs[:], func=Act.Tanh)
            hs = H[s]
            nc.vector.tensor_tensor(out=hs[:], in0=SO[:], in1=T[:], op=Alu.mult)

            nc.sync.dma_start(out=c_traj[ds((jv * SPG + s) * 128, 128), :], in_=cs[:])
            nc.sync.dma_start(out=h_traj[ds((jv * SPG + s) * 128, 128), :], in_=hs[:])
            # After the last step of group j has read its ST slice, prefetch
            # that slice for the next block.
            if u == SPG - 1 and gather:
                gather_xp(j)

        # The loop runs all blocks except the last; the unrolled epilogue
        # block issues no gathers or token DMAs, so the DMA queues drain
        # during (not after) the final 64 steps.
        with tc.For_i(
            0, (seq - body) // SPG, body // SPG,
            hint_engines=(mybir.EngineType.PE,), staggered_reset=False,
        ) as jv:
            # Stage the NEXT block's token columns; in-loop gathers prefetch
            # for block i+1 while this block computes.
            nc.sync.dma_start(out=tok_cur[:], in_=tok4[:, ds(jv + ngrp, ngrp)])
            for s in range(body):
                step(jv, s)
        for s in range(body):
            step((seq - body) // SPG, s, gather=False)

    nc.finalize()
    return nc


def _get_program():
    global _PROGRAM
    if _PROGRAM is None:
        _PROGRAM = _build_program()
    return _PROGRAM


def _prep_host(inputs, Wi, Wh, b):
    tokens = np.argmax(inputs, axis=-1).astype(np.int32)  # [B, T]
    eos = inputs[:, :, EOS_ID] > 0.5
    any_eos = eos.any(axis=1)
    t_star = np.where(any_eos, eos.argmax(axis=1), SEQ - 1).astype(np.int64)

    # Gate reorder (g, i, f, o): gates whose results are needed earliest in
    # the elementwise chain close their PSUM banks first; o closes last and
    # has the shortest tail (h = sig(o) * tanh(c)).
    perm = np.concatenate(
        [np.arange(1024, 1536), np.arange(0, 512), np.arange(512, 1024), np.arange(1536, 2048)]
    )
    Wi_re = (Wi.astype(np.float32) + b.astype(np.float32)[None, :])[:, perm]
    Wh_re = Wh.astype(np.float32)[:, perm]

    # wi rows stay in z-feature order (gate-permuted only); wh tile k*16+t
    # holds Wh_re[128k:128k+128, 128t:128t+128], stored partition-major.
    Wi_dev = np.ascontiguousarray(Wi_re).astype(np.float16)
    # Partition-major: wh[kr, (k*NT+t)*128 + p] = Wh_re[128k+kr, 128t+p]
    Wh_dev = np.ascontiguousarray(
        Wh_re.reshape(NK, 128, NT, 128).transpose(1, 0, 2, 3).reshape(128, NK * NT * 128)
    ).astype(np.float16)
    return tokens, t_star, Wi_dev, Wh_dev


def _pack4(arr16):
    """[16, 4*n, ...] -> [128, n, ...] with step 4j+u at partition 32u+b."""
    n4 = arr16.shape[1]
    n = n4 // SPG
    out = np.zeros((128, n) + arr16.shape[2:], arr16.dtype)
    for u in range(SPG):
        out[32 * u : 32 * u + BLOC] = arr16[:, u::SPG]
    return out


def kernel(inputs, Wi, Wh, b):
    global LAST_RESULTS
    inputs = np.asarray(inputs)
    Wi = np.asarray(Wi)
    Wh = np.asarray(Wh)
    b = np.asarray(b)

    tokens, t_star, Wi_dev, Wh_dev = _prep_host(inputs, Wi, Wh, b)

    id_rep = np.zeros((128, BLOC), np.float16)
    for u in range(SPG):
        id_rep[32 * u : 32 * u + BLOC] = np.eye(BLOC, dtype=np.float16)

    in_maps = []
    for n in range(NCORES):
        tokc = tokens[BLOC * n : BLOC * (n + 1)]
        tok_pad = np.concatenate([tokc, np.zeros((BLOC, BODY), np.int32)], axis=1)
        tok4 = _pack4(tok_pad)
        st0 = _pack4(Wi_dev[tokc[:, 0:BODY]]).reshape(128, (BODY // SPG) * GATES)
        in_maps.append(
            {
                "wi": Wi_dev,
                "wh": Wh_dev,
                "tok4": np.ascontiguousarray(tok4),
                "ident": id_rep,
                "st0": np.ascontiguousarray(st0),
            }
        )

    nc = _get_program()
    res = run_bass_kernel_spmd(nc, in_maps, list(range(NCORES)), trace=TRACE)
    LAST_RESULTS = res

    c_out = np.zeros((BATCH, HID), np.float32)
    h_out = np.zeros((BATCH, HID), np.float32)
    for n in range(NCORES):
        ct = res.results[n]["c_traj"].reshape(SEQ, 128, 64).astype(np.float32)
        ht = res.results[n]["h_traj"].reshape(SEQ, 128, 64).astype(np.float32)
        for bl in range(BLOC):
            g = BLOC * n + bl
            t = int(t_star[g])
            c_out[g] = ct[t][:, bl::BLOC].T.reshape(HID)
            h_out[g] = ht[t][:, bl::BLOC].T.reshape(HID)
    return (c_out, h_out)


# revision 32
# speedup vs baseline: 1.0229x; 1.0220x over previous
"""LSTM encoder with EOS-freeze for Trainium2, data-parallel over batch on 8 cores.

Strategy
--------
Inputs are one-hot, so x @ Wi is a row-gather of Wi done with indirect DMA on
device. The recurrent h @ Wh runs on the tensor engine with Wh as 64 fp16
[128,128] stationary tiles and h.T chunks as the [128,16] moving operand,
producing z transposed: PSUM [128 partitions = z-feature % 128, 16*tile + b].
Gates are reordered (g, i, f, o) host-side, one PSUM bank per gate, so each
gate's activation starts as soon as its own 16 recurrent matmuls close instead
of waiting for all 64 — the elementwise chain overlaps the tensor engine.

Gathered x rows are packed 4 steps per 2048-column slice at partition offsets
{0,32,64,96} (the valid PE tile_position row bases for a 16-row stationary).
This spreads gather/preload DMAs across 64 partitions instead of 16 (4x DMA
bandwidth), shrinks SBUF so a 64-step loop body fits (two For_i barriers plus
an unrolled epilogue block instead of 15 boundaries), and batches the
per-block gathers into 16 DGE instructions. Block 0's rows are gathered
host-side and loaded with a few direct DMAs (smallest chunk first) so the
gpsimd queue is empty before the loop-entry barrier and the first matmuls
start ~13us into the kernel. The epilogue block issues no gathers, letting
the DMA queues drain during, not after, the final 64 steps.

The EOS freeze is handled without any per-step masking: sequences are
independent, so the kernel runs the unmasked recurrence and streams per-step
(c, h) snapshots to DRAM; the frozen value for sequence b is the snapshot at
its first-EOS step, selected during unshard.
"""

import numpy as np

try:
    import concourse  # noqa: F401
except ImportError:
    import sys

    sys.path.insert(0, "/opt/trn_rl_repo")

from contextlib import ExitStack

import concourse.bass as bass
import concourse.tile as tile
from concourse import bacc
from concourse import mybir
from concourse.bass import ds
from concourse.bass_utils import run_bass_kernel_spmd

dt = mybir.dt
Alu = mybir.AluOpType
Act = mybir.ActivationFunctionType

EOS_ID = 1
HID = 512
BATCH, SEQ, VOCAB = 128, 256, 1024
GATES = 4 * HID  # 2048
NCORES = 8
BLOC = BATCH // NCORES  # 16 sequences per core
NT = GATES // 128  # 16 feature tiles of z
NK = HID // 128  # 4 contraction chunks
SPG = 4  # steps packed per ST column-slice (partition offsets 0/32/64/96)
BODY = 64  # steps per For_i iteration
NGRP = BODY // SPG  # gather groups per block

# Collect profiling info when True (set by test.py; adds trace overhead).
TRACE = False
LAST_RESULTS = None  # BassKernelResults of the last run, for test.py

_PROGRAM = None


def _build_program(seq=SEQ, body=BODY):
    ngrp = body // SPG
    nc = bacc.Bacc("TRN2", debug=False, detect_race_conditions=False)

    wi = nc.declare_dram_parameter("wi", [VOCAB, GATES], dt.float16, isOutput=False)
    ident = nc.declare_dram_parameter("ident", [128, BLOC], dt.float16, isOutput=False)
    wh = nc.declare_dram_parameter("wh", [128, NK * NT * 128], dt.float16, isOutput=False)
    # tok4[32u+b, j] = token of sequence b at step 4j+u (b<16; other rows 0).
    tok4 = nc.declare_dram_parameter(
        "tok4", [128, (seq + body) // SPG], dt.int32, isOutput=False
    )
    # Block 0's gathered Wi rows, prepared host-side (one direct DMA).
    st0 = nc.declare_dram_parameter("st0", [128, ngrp * GATES], dt.float16, isOutput=False)
    c_traj = nc.declare_dram_parameter("c_traj", [seq * 128, 64], dt.float16, isOutput=True)
    h_traj = nc.declare_dram_parameter("h_traj", [seq * 128, 64], dt.float16, isOutput=True)

    with tile.TileContext(nc) as tc, ExitStack() as ctx:
        pool = lambda name, bufs, **kw: ctx.enter_context(
            tc.tile_pool(name=name, bufs=bufs, **kw)
        )
        whp = pool("whp", 1)
        tokp = pool("tokp", 1)
        stp = pool("stp", 1)
        hp = pool("hp", 1)
        cp = pool("cp", 1)
        zp_pool = pool("zp", 2, space="PSUM")
        sp = pool("sp", 2)
        gp = pool("gp", 2)
        ap_ = pool("ap", 2)
        bp = pool("bp", 2)
        tp = pool("tp", 2)

        # One big tile holding all gather groups; subtile dep tracking scopes
        # reads/writes to the per-group 2048-column slices. Load block 0 in a
        # few chunks, smallest first and ahead of the 2MB wh transfer, so the
        # first steps' matmuls start as early as possible — but no more DMAs
        # than free DMA semaphores, or the sync queue blocks on semaphore
        # reuse before the loop-entry barrier.
        ST = stp.tile([128, ngrp * GATES], dt.float16, name="st_all")
        id_sb = tokp.tile([128, BLOC], dt.float16, name="id_sb")
        nc.sync.dma_start(out=id_sb[:], in_=ident[:, :])
        # Split group 0 in half: gates g,i of steps 0-3 arrive first so the
        # earliest x-matmuls start ~2.5us sooner.
        nc.sync.dma_start(out=ST[:, 0 : GATES // 2], in_=st0[:, 0 : GATES // 2])
        nc.sync.dma_start(out=ST[:, GATES // 2 : GATES], in_=st0[:, GATES // 2 : GATES])
        wh_sb = whp.tile([128, NK * NT * 128], dt.float16, name="wh_sb")
        nc.sync.dma_start(out=wh_sb[:], in_=wh[:, :])
        # Fully unrolled program: the whole token table fits in 320B per
        # partition, so load it once and index gathers statically.
        tok_all = tokp.tile([128, (seq + body) // SPG], dt.int32, name="tok_all")
        nc.sync.dma_start(out=tok_all[:], in_=tok4[:, :])
        bounds = [1, 2, (ngrp + 2) // 2, ngrp]
        for j0, j1 in zip(bounds, bounds[1:]):
            nc.sync.dma_start(
                out=ST[:, j0 * GATES : j1 * GATES],
                in_=st0[:, j0 * GATES : j1 * GATES],
            )
        H = [hp.tile([128, 64], dt.float16, name=f"h{s}", tag=f"h{s}") for s in range(body)]
        C = [cp.tile([128, 64], dt.float16, name=f"c{s}", tag=f"c{s}") for s in range(body)]

        nc.vector.memset(H[body - 1][:], 0.0)
        nc.vector.memset(C[body - 1][:], 0.0)

        def gather_xp(j_abs):
            # Gather wi rows for absolute group j_abs (4 steps x 16 sequences;
            # rows 16..31 of each 32-partition group are dummy token 0) into
            # its ST slot — one row per partition, the DGE-supported shape.
            j = j_abs % ngrp
            nc.gpsimd.indirect_dma_start(
                out=ST[:, j * GATES : (j + 1) * GATES],
                out_offset=None,
                in_=wi[:, :],
                in_offset=bass.IndirectOffsetOnAxis(ap=tok_all[:, j_abs : j_abs + 1], axis=0),
            )

        def step(blk, s, gather=True):
            u, j = s % SPG, s // SPG
            hprev = H[(s - 1) % body]
            cprev = C[(s - 1) % body]
            # One PSUM bank per gate so each gate's accumulation group closes
            # after its own 16 matmuls and its activation overlaps the
            # remaining gates' matmuls. Gate order in z columns: g, i, f, o.
            Z = [
                zp_pool.tile([128, 64], dt.float32, name=f"z{q}", tag=f"z{q}")
                for q in range(4)
            ]
            # x@Wi enters PSUM via PE transpose of the gathered rows: these
            # matmuls need no h, so they overlap the previous step's tail.
            for q in range(4):
                for t in range(4):
                    tg = 4 * q + t
                    # start=True on the first matmul clears the bank's
                    # has_written bits; later matmuls join the group.
                    nc.tensor.matmul(
                        out=Z[q][:, 16 * t : 16 * t + 16],
                        lhsT=ST[
                            32 * u : 32 * u + BLOC,
                            j * GATES + 128 * tg : j * GATES + 128 * tg + 128,
                        ],
                        rhs=id_sb[32 * u : 32 * u + BLOC, :],
                        start=(t == 0),
                        stop=False,
                        tile_position=(32 * u, 0),
                    )
            for q in range(4):
                for t in range(4):
                    tg = 4 * q + t
                    for k in range(NK):
                        nc.tensor.matmul(
                            out=Z[q][:, 16 * t : 16 * t + 16],
                            lhsT=wh_sb[:, (k * NT + tg) * 128 : (k * NT + tg) * 128 + 128],
                            rhs=hprev[:, 16 * k : 16 * k + 16],
                            start=False,
                            stop=(t == 3 and k == NK - 1),
                        )
            TG = gp.tile([128, 64], dt.float16, name="TG", tag="TG")
            nc.scalar.activation(out=TG[:], in_=Z[0][:], func=Act.Tanh)
            SI = sp.tile([128, 64], dt.float16, name="SI", tag="SI")
            nc.scalar.activation(out=SI[:], in_=Z[1][:], func=Act.Sigmoid)
            SF = sp.tile([128, 64], dt.float16, name="SF", tag="SF")
            nc.scalar.activation(out=SF[:], in_=Z[2][:], func=Act.Sigmoid)
            SO = sp.tile([128, 64], dt.float16, name="SO", tag="SO")
            nc.scalar.activation(out=SO[:], in_=Z[3][:], func=Act.Sigmoid)
            A = ap_.tile([128, 64], dt.float16, name="A", tag="A")
            nc.vector.tensor_tensor(out=A[:], in0=SI[:], in1=TG[:], op=Alu.mult)
            B = bp.tile([128, 64], dt.float16, name="B", tag="B")
            nc.vector.tensor_tensor(out=B[:], in0=SF[:], in1=cprev[:], op=Alu.mult)
            cs = C[s]
            nc.vector.tensor_tensor(out=cs[:], in0=A[:], in1=B[:], op=Alu.add)
            T = tp.tile([128, 64], dt.float16, name="T", tag="T")
            nc.scalar.activation(out=T[:], in_=cs[:], func=Act.Tanh)
            hs = H[s]
            nc.vector.tensor_tensor(out=hs[:], in0=SO[:], in1=T[:], op=Alu.mult)

            nc.sync.dma_start(out=c_traj[ds((blk * body + s) * 128, 128), :], in_=cs[:])
            nc.sync.dma_start(out=h_traj[ds((blk * body + s) * 128, 128), :], in_=hs[:])
            # After the last step of group j has read its ST slice, prefetch
            # that slice for the next block.
            if u == SPG - 1 and gather:
                gather_xp((blk + 1) * ngrp + j)

        # Fully unrolled: no For_i means no per-iteration all-engine barriers
        # or semaphore-reset stalls; the last block issues no gathers so the
        # DMA queues drain during (not after) the final 64 steps.
        nblk = seq // body
        for blk in range(nblk):
            for s in range(body):
                step(blk, s, gather=(blk < nblk - 1))

    nc.finalize()
    return nc


def _get_program():
    global _PROGRAM
    if _PROGRAM is None:
        _PROGRAM = _build_program()
    return _PROGRAM


def _prep_host(inputs, Wi, Wh, b):
    tokens = np.argmax(inputs, axis=-1).astype(np.int32)  # [B, T]
    eos = inputs[:, :, EOS_ID] > 0.5
    any_eos = eos.any(axis=1)
    t_star = np.where(any_eos, eos.argmax(axis=1), SEQ - 1).astype(np.int64)

    # Gate reorder (g, i, f, o): gates whose results are needed earliest in
    # the elementwise chain close their PSUM banks first; o closes last and
    # has the shortest tail (h = sig(o) * tanh(c)).
    perm = np.concatenate(
        [np.arange(1024, 1536), np.arange(0, 512), np.arange(512, 1024), np.arange(1536, 2048)]
    )
    Wi_re = (Wi.astype(np.float32) + b.astype(np.float32)[None, :])[:, perm]
    Wh_re = Wh.astype(np.float32)[:, perm]

    # wi rows stay in z-feature order (gate-permuted only); wh tile k*16+t
    # holds Wh_re[128k:128k+128, 128t:128t+128], stored partition-major.
    Wi_dev = np.ascontiguousarray(Wi_re).astype(np.float16)
    # Partition-major: wh[kr, (k*NT+t)*128 + p] = Wh_re[128k+kr, 128t+p]
    Wh_dev = np.ascontiguousarray(
        Wh_re.reshape(NK, 128, NT, 128).transpose(1, 0, 2, 3).reshape(128, NK * NT * 128)
    ).astype(np.float16)
    return tokens, t_star, Wi_dev, Wh_dev


def _pack4(arr16):
    """[16, 4*n, ...] -> [128, n, ...] with step 4j+u at partition 32u+b."""
    n4 = arr16.shape[1]
    n = n4 // SPG
    out = np.zeros((128, n) + arr16.shape[2:], arr16.dtype)
    for u in range(SPG):
        out[32 * u : 32 * u + BLOC] = arr16[:, u::SPG]
    return out


def kernel(inputs, Wi, Wh, b):
    global LAST_RESULTS
    inputs = np.asarray(inputs)
    Wi = np.asarray(Wi)
    Wh = np.asarray(Wh)
    b = np.asarray(b)

    tokens, t_star, Wi_dev, Wh_dev = _prep_host(inputs, Wi, Wh, b)

    id_rep = np.zeros((128, BLOC), np.float16)
    for u in range(SPG):
        id_rep[32 * u : 32 * u + BLOC] = np.eye(BLOC, dtype=np.float16)

    in_maps = []
    for n in range(NCORES):
        tokc = tokens[BLOC * n : BLOC * (n + 1)]
        tok_pad = np.concatenate([tokc, np.zeros((BLOC, BODY), np.int32)], axis=1)
        tok4 = _pack4(tok_pad)
        st0 = _pack4(Wi_dev[tokc[:, 0:BODY]]).reshape(128, (BODY // SPG) * GATES)
        in_maps.append(
            {
                "wi": Wi_dev,
                "wh": Wh_dev,
                "tok4": np.ascontiguousarray(tok4),
                "ident": id_rep,
                "st0": np.ascontiguousarray(st0),
            }
        )

    nc = _get_program()
    res = run_bass_kernel_spmd(nc, in_maps, list(range(NCORES)), trace=TRACE)
    LAST_RESULTS = res

    c_out = np.zeros((BATCH, HID), np.float32)
    h_out = np.zeros((BATCH, HID), np.float32)
    for n in range(NCORES):
        ct = res.results[n]["c_traj"].reshape(SEQ, 128, 64).astype(np.float32)
        ht = res.results[n]["h_traj"].reshape(SEQ, 128, 64).astype(np.float32)
        for bl in range(BLOC):
            g = BLOC * n + bl
            t = int(t_star[g])
            c_out[g] = ct[t][:, bl::BLOC].T.reshape(HID)
            h_out[g] = ht[t][:, bl::BLOC].T.reshape(HID)
    return (c_out, h_out)


# revision 36
# speedup vs baseline: 1.0245x; 1.0016x over previous
"""LSTM encoder with EOS-freeze for Trainium2, data-parallel over batch on 8 cores.

Strategy
--------
Inputs are one-hot, so x @ Wi is a row-gather of Wi done with indirect DMA on
device. The recurrent h @ Wh runs on the tensor engine with Wh as 64 fp16
[128,128] stationary tiles and h.T chunks as the [128,16] moving operand,
producing z transposed: PSUM [128 partitions = z-feature % 128, 16*tile + b].
Gates are reordered (g, i, f, o) host-side, one PSUM bank per gate, so each
gate's activation starts as soon as its own 16 recurrent matmuls close instead
of waiting for all 64 — the elementwise chain overlaps the tensor engine.

Gathered x rows are packed 4 steps per 2048-column slice at partition offsets
{0,32,64,96} (the valid PE tile_position row bases for a 16-row stationary).
This spreads gather/preload DMAs across 64 partitions instead of 16 (4x DMA
bandwidth), shrinks SBUF so a 64-step block of slots fits, and batches the
per-block gathers into 16 DGE instructions. The whole 256-step program is
fully unrolled — no For_i, so no per-iteration all-engine barriers or
semaphore-reset stalls — with the tiny token table resident in SBUF. Block
0's rows are gathered host-side and loaded with a few direct DMAs (smallest
chunk first) so the first matmuls start ~13us into the kernel; the last block
issues no gathers, letting the DMA queues drain during the final 64 steps.

The EOS freeze is handled without any per-step masking: sequences are
independent, so the kernel runs the unmasked recurrence and streams per-step
(c, h) snapshots to DRAM; the frozen value for sequence b is the snapshot at
its first-EOS step, selected during unshard.
"""

import numpy as np

try:
    import concourse  # noqa: F401
except ImportError:
    import sys

    sys.path.insert(0, "/opt/trn_rl_repo")

from contextlib import ExitStack

import concourse.bass as bass
import concourse.tile as tile
from concourse import bacc
from concourse import mybir
from concourse.bass import ds
from concourse.bass_utils import run_bass_kernel_spmd

dt = mybir.dt
Alu = mybir.AluOpType
Act = mybir.ActivationFunctionType

EOS_ID = 1
HID = 512
BATCH, SEQ, VOCAB = 128, 256, 1024
GATES = 4 * HID  # 2048
NCORES = 8
BLOC = BATCH // NCORES  # 16 sequences per core
NT = GATES // 128  # 16 feature tiles of z
NK = HID // 128  # 4 contraction chunks
SPG = 4  # steps packed per ST column-slice (partition offsets 0/32/64/96)
BODY = 64  # steps per For_i iteration
NGRP = BODY // SPG  # gather groups per block

# Collect profiling info when True (set by test.py; adds trace overhead).
TRACE = False
LAST_RESULTS = None  # BassKernelResults of the last run, for test.py

_PROGRAM = None


def _build_program(seq=SEQ, body=BODY):
    ngrp = body // SPG
    nc = bacc.Bacc("TRN2", debug=False, detect_race_conditions=False)

    wi = nc.declare_dram_parameter("wi", [VOCAB, GATES], dt.float16, isOutput=False)
    ident = nc.declare_dram_parameter("ident", [128, BLOC], dt.float16, isOutput=False)
    wh = nc.declare_dram_parameter("wh", [128, NK * NT * 128], dt.float16, isOutput=False)
    # tok4[32u+b, j] = token of sequence b at step 4j+u (b<16; other rows 0).
    tok4 = nc.declare_dram_parameter(
        "tok4", [128, (seq + body) // SPG], dt.int32, isOutput=False
    )
    # Block 0's gathered Wi rows, prepared host-side (one direct DMA).
    st0 = nc.declare_dram_parameter("st0", [128, ngrp * GATES], dt.float16, isOutput=False)
    c_traj = nc.declare_dram_parameter("c_traj", [seq * 128, 64], dt.float16, isOutput=True)
    h_traj = nc.declare_dram_parameter("h_traj", [seq * 128, 64], dt.float16, isOutput=True)

    with tile.TileContext(nc) as tc, ExitStack() as ctx:
        pool = lambda name, bufs, **kw: ctx.enter_context(
            tc.tile_pool(name=name, bufs=bufs, **kw)
        )
        whp = pool("whp", 1)
        tokp = pool("tokp", 1)
        stp = pool("stp", 1)
        hp = pool("hp", 1)
        cp = pool("cp", 1)
        zp_pool = pool("zp", 2, space="PSUM")
        sp = pool("sp", 2)
        gp = pool("gp", 2)
        ap_ = pool("ap", 2)
        bp = pool("bp", 2)
        tp = pool("tp", 2)

        # One big tile holding all gather groups; subtile dep tracking scopes
        # reads/writes to the per-group 2048-column slices. Load block 0 in a
        # few chunks, smallest first and ahead of the 2MB wh transfer, so the
        # first steps' matmuls start as early as possible — but no more DMAs
        # than free DMA semaphores, or the sync queue blocks on semaphore
        # reuse before the loop-entry barrier.
        ST = stp.tile([128, ngrp * GATES], dt.float16, name="st_all")
        id_sb = tokp.tile([128, BLOC], dt.float16, name="id_sb")
        nc.sync.dma_start(out=id_sb[:], in_=ident[:, :])
        # Split group 0 in half: gates g,i of steps 0-3 arrive first so the
        # earliest x-matmuls start ~2.5us sooner.
        nc.sync.dma_start(out=ST[:, 0 : GATES // 2], in_=st0[:, 0 : GATES // 2])
        nc.sync.dma_start(out=ST[:, GATES // 2 : GATES], in_=st0[:, GATES // 2 : GATES])
        # wh is laid out gate-major (2048-column block per gate) so it can
        # stream in per-gate DMAs ordered by first use: step 1's gate-g
        # matmuls only wait for the first 512KB, not the whole 2MB.
        wh_sb = whp.tile([128, NK * NT * 128], dt.float16, name="wh_sb")
        for q in range(4):
            nc.sync.dma_start(
                out=wh_sb[:, q * 4 * NK * 128 : (q + 1) * 4 * NK * 128],
                in_=wh[:, q * 4 * NK * 128 : (q + 1) * 4 * NK * 128],
            )
        # Fully unrolled program: the whole token table fits in 320B per
        # partition, so load it once and index gathers statically.
        tok_all = tokp.tile([128, (seq + body) // SPG], dt.int32, name="tok_all")
        nc.sync.dma_start(out=tok_all[:], in_=tok4[:, :])
        bounds = [1, 2, (ngrp + 2) // 2, ngrp]
        for j0, j1 in zip(bounds, bounds[1:]):
            nc.sync.dma_start(
                out=ST[:, j0 * GATES : j1 * GATES],
                in_=st0[:, j0 * GATES : j1 * GATES],
            )
        H = [hp.tile([128, 64], dt.float16, name=f"h{s}", tag=f"h{s}") for s in range(body)]
        C = [cp.tile([128, 64], dt.float16, name=f"c{s}", tag=f"c{s}") for s in range(body)]

        nc.vector.memset(H[body - 1][:], 0.0)
        nc.vector.memset(C[body - 1][:], 0.0)

        def gather_xp(j_abs):
            # Gather wi rows for absolute group j_abs (4 steps x 16 sequences;
            # rows 16..31 of each 32-partition group are dummy token 0) into
            # its ST slot — one row per partition, the DGE-supported shape.
            j = j_abs % ngrp
            nc.gpsimd.indirect_dma_start(
                out=ST[:, j * GATES : (j + 1) * GATES],
                out_offset=None,
                in_=wi[:, :],
                in_offset=bass.IndirectOffsetOnAxis(ap=tok_all[:, j_abs : j_abs + 1], axis=0),
            )

        def step(blk, s, gather=True):
            u, j = s % SPG, s // SPG
            hprev = H[(s - 1) % body]
            cprev = C[(s - 1) % body]
            # One PSUM bank per gate so each gate's accumulation group closes
            # after its own 16 matmuls and its activation overlaps the
            # remaining gates' matmuls. Gate order in z columns: g, i, f, o.
            Z = [
                zp_pool.tile([128, 64], dt.float32, name=f"z{q}", tag=f"z{q}")
                for q in range(4)
            ]
            # x@Wi enters PSUM via PE transpose of the gathered rows: these
            # matmuls need no h, so they overlap the previous step's tail.
            for q in range(4):
                for t in range(4):
                    tg = 4 * q + t
                    # start=True on the first matmul clears the bank's
                    # has_written bits; later matmuls join the group.
                    nc.tensor.matmul(
                        out=Z[q][:, 16 * t : 16 * t + 16],
                        lhsT=ST[
                            32 * u : 32 * u + BLOC,
                            j * GATES + 128 * tg : j * GATES + 128 * tg + 128,
                        ],
                        rhs=id_sb[32 * u : 32 * u + BLOC, :],
                        start=(t == 0),
                        stop=False,
                        tile_position=(32 * u, 0),
                    )
            for q in range(4):
                for t in range(4):
                    tg = 4 * q + t
                    for k in range(NK):
                        wc = ((q * 4 + t) * NK + k) * 128
                        nc.tensor.matmul(
                            out=Z[q][:, 16 * t : 16 * t + 16],
                            lhsT=wh_sb[:, wc : wc + 128],
                            rhs=hprev[:, 16 * k : 16 * k + 16],
                            start=False,
                            stop=(t == 3 and k == NK - 1),
                        )
            TG = gp.tile([128, 64], dt.float16, name="TG", tag="TG")
            nc.scalar.activation(out=TG[:], in_=Z[0][:], func=Act.Tanh)
            SI = sp.tile([128, 64], dt.float16, name="SI", tag="SI")
            nc.scalar.activation(out=SI[:], in_=Z[1][:], func=Act.Sigmoid)
            SF = sp.tile([128, 64], dt.float16, name="SF", tag="SF")
            nc.scalar.activation(out=SF[:], in_=Z[2][:], func=Act.Sigmoid)
            SO = sp.tile([128, 64], dt.float16, name="SO", tag="SO")
            nc.scalar.activation(out=SO[:], in_=Z[3][:], func=Act.Sigmoid)
            A = ap_.tile([128, 64], dt.float16, name="A", tag="A")
            nc.vector.tensor_tensor(out=A[:], in0=SI[:], in1=TG[:], op=Alu.mult)
            B = bp.tile([128, 64], dt.float16, name="B", tag="B")
            nc.vector.tensor_tensor(out=B[:], in0=SF[:], in1=cprev[:], op=Alu.mult)
            cs = C[s]
            nc.vector.tensor_tensor(out=cs[:], in0=A[:], in1=B[:], op=Alu.add)
            T = tp.tile([128, 64], dt.float16, name="T", tag="T")
            nc.scalar.activation(out=T[:], in_=cs[:], func=Act.Tanh)
            hs = H[s]
            nc.vector.tensor_tensor(out=hs[:], in0=SO[:], in1=T[:], op=Alu.mult)

            nc.sync.dma_start(out=c_traj[ds((blk * body + s) * 128, 128), :], in_=cs[:])
            nc.sync.dma_start(out=h_traj[ds((blk * body + s) * 128, 128), :], in_=hs[:])
            # After the last step of group j has read its ST slice, prefetch
            # that slice for the next block.
            if u == SPG - 1 and gather:
                gather_xp((blk + 1) * ngrp + j)

        # Fully unrolled: no For_i means no per-iteration all-engine barriers
        # or semaphore-reset stalls; the last block issues no gathers so the
        # DMA queues drain during (not after) the final 64 steps.
        nblk = seq // body
        for blk in range(nblk):
            for s in range(body):
                step(blk, s, gather=(blk < nblk - 1))

    nc.finalize()
    return nc


def _get_program():
    global _PROGRAM
    if _PROGRAM is None:
        _PROGRAM = _build_program()
    return _PROGRAM


def _prep_host(inputs, Wi, Wh, b):
    tokens = np.argmax(inputs, axis=-1).astype(np.int32)  # [B, T]
    eos = inputs[:, :, EOS_ID] > 0.5
    any_eos = eos.any(axis=1)
    t_star = np.where(any_eos, eos.argmax(axis=1), SEQ - 1).astype(np.int64)

    # Gate reorder (g, i, f, o): gates whose results are needed earliest in
    # the elementwise chain close their PSUM banks first; o closes last and
    # has the shortest tail (h = sig(o) * tanh(c)).
    perm = np.concatenate(
        [np.arange(1024, 1536), np.arange(0, 512), np.arange(512, 1024), np.arange(1536, 2048)]
    )
    Wi_re = (Wi.astype(np.float32) + b.astype(np.float32)[None, :])[:, perm]
    Wh_re = Wh.astype(np.float32)[:, perm]

    # wi rows stay in z-feature order (gate-permuted only); wh tile k*16+t
    # holds Wh_re[128k:128k+128, 128t:128t+128], stored partition-major.
    Wi_dev = np.ascontiguousarray(Wi_re).astype(np.float16)
    # Gate-major: wh[kr, (t*NK+k)*128 + p] = Wh_re[128k+kr, 128t+p] — each
    # gate's 4 feature tiles x 4 k-chunks occupy one contiguous 2048-column
    # block, so wh can stream in per-gate DMAs.
    Wh_dev = np.ascontiguousarray(
        Wh_re.reshape(NK, 128, NT, 128).transpose(1, 2, 0, 3).reshape(128, NK * NT * 128)
    ).astype(np.float16)
    return tokens, t_star, Wi_dev, Wh_dev


def _pack4(arr16):
    """[16, 4*n, ...] -> [128, n, ...] with step 4j+u at partition 32u+b."""
    n4 = arr16.shape[1]
    n = n4 // SPG
    out = np.zeros((128, n) + arr16.shape[2:], arr16.dtype)
    for u in range(SPG):
        out[32 * u : 32 * u + BLOC] = arr16[:, u::SPG]
    return out


def kernel(inputs, Wi, Wh, b):
    global LAST_RESULTS
    inputs = np.asarray(inputs)
    Wi = np.asarray(Wi)
    Wh = np.asarray(Wh)
    b = np.asarray(b)

    tokens, t_star, Wi_dev, Wh_dev = _prep_host(inputs, Wi, Wh, b)

    id_rep = np.zeros((128, BLOC), np.float16)
    for u in range(SPG):
        id_rep[32 * u : 32 * u + BLOC] = np.eye(BLOC, dtype=np.float16)

    in_maps = []
    for n in range(NCORES):
        tokc = tokens[BLOC * n : BLOC * (n + 1)]
        tok_pad = np.concatenate([tokc, np.zeros((BLOC, BODY), np.int32)], axis=1)
        tok4 = _pack4(tok_pad)
        st0 = _pack4(Wi_dev[tokc[:, 0:BODY]]).reshape(128, (BODY // SPG) * GATES)
        in_maps.append(
            {
                "wi": Wi_dev,
                "wh": Wh_dev,
                "tok4": np.ascontiguousarray(tok4),
                "ident": id_rep,
                "st0": np.ascontiguousarray(st0),
            }
        )

    nc = _get_program()
    res = run_bass_kernel_spmd(nc, in_maps, list(range(NCORES)), trace=TRACE)
    LAST_RESULTS = res

    c_out = np.zeros((BATCH, HID), np.float32)
    h_out = np.zeros((BATCH, HID), np.float32)
    for n in range(NCORES):
        ct = res.results[n]["c_traj"].reshape(SEQ, 128, 64).astype(np.float32)
        ht = res.results[n]["h_traj"].reshape(SEQ, 128, 64).astype(np.float32)
        for bl in range(BLOC):
            g = BLOC * n + bl
            t = int(t_star[g])
            c_out[g] = ct[t][:, bl::BLOC].T.reshape(HID)
            h_out[g] = ht[t][:, bl::BLOC].T.reshape(HID)
    return (c_out, h_out)
